# revision 17
# baseline (speedup 1.0000x reference)
"""Trainium2 Bass kernel for causal self-attention with T5 relative position bias.

Problem (hardcoded): B=4, T=2048, C=1024, H=16, D=64, NUM_BUCKETS=32, MAX_DISTANCE=128.
Sharding over 8 cores: core c -> (batch b=c//2, head-group hg=c%2 of 8 heads).
Each core computes qkv projection for its heads, causal attention, and a partial
output projection (its heads' rows of W_proj); host sums the two partials per batch.

Key structure (v2):
  - Heads are processed in PAIRS (2m on partitions 0-63, 2m+1 on 64-127). The
    two K=64 S-matmuls of a pair target PE row-groups (0,0) and (64,0), so the
    tensor engine runs them CONCURRENTLY (row tiling).
  - AV is "flipped": P tiles [tk,128tq] are the stationary operand, v+ones
    [tk,65] the moving operand -> out [tq,65] costs 65 cycles instead of 512,
    and the softmax rowsum lands as a per-partition COLUMN, so normalization is
    a parallel DVE reciprocal + tensor_scalar multiply (no DRAM bounce).
  - The T5 bias table is host-divided by exp(b31) (bucket 31 = all d >= 106),
    which makes the table exactly 1.0 for far tiles: those skip the DVE
    mask-multiply entirely, and the per-head exp(b31) factor cancels in the
    softmax ratio.
  - Normalized y tiles [tq,128] (head pair side by side) are PE-transposed back
    to [c',tq] for the output projection.
  - Emission is chunk-pipelined: qkv of chunk c+1 and proj of chunk c-1 are
    emitted as PE fillers between S and AV inside attention of chunk c, so the
    tensor engine streams while the scalar engine runs the exps.
"""

import sys

sys.path.insert(0, "/opt/trn_rl_repo")

import math
from collections import deque

import numpy as np

import concourse.bacc as bacc
import concourse.bass as bass
import concourse.mybir as mybir
import concourse.tile as tile
from concourse import bass_utils


def _ensure_axon_hooks():
    """bass_utils imports antenv.axon_hooks when BASS_TRACE is set under axon;
    this image's antenv lacks that submodule. Provide an inert one so a stray
    trace env var degrades to a warning instead of crashing the run."""
    try:
        import antenv.axon_hooks  # noqa: F401
    except Exception:
        try:
            import types

            import antenv

            hooks = types.ModuleType("antenv.axon_hooks")
            hooks._hook = None
            hooks.set_axon_ntff_profile_hook = lambda h: setattr(hooks, "_hook", h)
            hooks.get_axon_ntff_profile_hook = lambda: hooks._hook
            sys.modules["antenv.axon_hooks"] = hooks
            antenv.axon_hooks = hooks
        except Exception:
            pass


_ensure_axon_hooks()

B, T, C = 4, 2048, 1024
H, D = 16, 64
NUM_BUCKETS, MAX_DISTANCE = 32, 128
HL = 8  # local heads per core
CL = HL * D  # 512 local channels
NCORES = 8
NPAIR = HL // 2  # 4 head pairs per core

FP16 = mybir.dt.float16
FP32 = mybir.dt.float32

NT = T // 512  # 4 tq chunks of 512
NK = T // 128  # 16 tk tiles of 128
KC = C // 128  # 8 contraction chunks for qkv
MC = CL // 128  # 4 m-chunks of local channels

# ea table geometry: slice start s = (tq0 - tk0) + 384 in [0, 1920], width 512
EA_W = 2432


def _build_program():
    nc = bacc.Bacc(None, target_bir_lowering=False)

    xT = nc.dram_tensor("xT", [C, T], FP16, kind="ExternalInput")
    wq = nc.dram_tensor("wq", [C, CL], FP16, kind="ExternalInput")
    wk = nc.dram_tensor("wk", [C, CL], FP16, kind="ExternalInput")
    wv = nc.dram_tensor("wv", [C, CL], FP16, kind="ExternalInput")
    wp = nc.dram_tensor("wp", [CL, C], FP16, kind="ExternalInput")
    bqk = nc.dram_tensor("bqk", [2, CL], FP32, kind="ExternalInput")
    bvr = nc.dram_tensor("bvr", [128, CL], FP32, kind="ExternalInput")
    # per-PAIR tables: [pair, head-in-pair, 128, EA_W], host-divided by exp(b31)
    wexp = nc.dram_tensor("wexp", [NPAIR, 2, 128, EA_W], FP16, kind="ExternalInput")
    ident = nc.dram_tensor("ident", [128, 128], FP16, kind="ExternalInput")
    yp = nc.dram_tensor("yp", [C, T], FP16, kind="ExternalOutput")

    import os

    DEBUG = os.environ.get("KDEBUG", "0") == "1"
    if DEBUG:
        d_q = nc.dram_tensor("d_q", [MC, 128, T], FP16, kind="ExternalOutput")
        d_k = nc.dram_tensor("d_k", [MC, 128, T], FP16, kind="ExternalOutput")
        d_v = nc.dram_tensor("d_v", [NK, 128, HL * 65], FP16, kind="ExternalOutput")
        d_y = nc.dram_tensor("d_y", [MC, 128, T], FP16, kind="ExternalOutput")
        d_ps = nc.dram_tensor("d_ps", [NT, 128, 1024], FP32, kind="ExternalOutput")
        d_pm = nc.dram_tensor("d_pm", [NT, 128, 1024], FP16, kind="ExternalOutput")
        d_pav = nc.dram_tensor("d_pav", [NT, 2, 128, 512], FP32, kind="ExternalOutput")

    with tile.TileContext(nc) as tc:
        with (
            tc.tile_pool(name="w", bufs=1) as wpool,
            tc.tile_pool(name="big", bufs=1) as bigpool,
            tc.tile_pool(name="ea", bufs=1) as eapool,
            tc.tile_pool(name="p2", bufs=3) as p2pool,
            tc.tile_pool(name="pm", bufs=3) as pmpool,
            tc.tile_pool(name="sm", bufs=2) as smpool,
            tc.tile_pool(name="yo", bufs=2) as yopool,
            tc.tile_pool(name="ps", bufs=2, space="PSUM") as ps,
            tc.tile_pool(name="pav", bufs=2, space="PSUM") as pavp,
            tc.tile_pool(name="misc", bufs=2, space="PSUM") as miscp,
        ):
            # ---- weights / constants ----
            wq_sb = wpool.tile([128, KC, CL], FP16)
            wk_sb = wpool.tile([128, KC, CL], FP16)
            wv_sb = wpool.tile([128, KC, CL], FP16)
            wp_sb = wpool.tile([128, MC, C], FP16)
            bq_sb = wpool.tile([128, MC], FP32)
            bk_sb = wpool.tile([128, MC], FP32)
            bv_sb = wpool.tile([128, CL], FP32)
            id_sb = wpool.tile([128, 128], FP16)
            xt_sb = bigpool.tile([128, KC, T], FP16)
            ea_sb = [
                eapool.tile([128, 2, EA_W], FP16, name=f"ea{p}") for p in range(NPAIR)
            ]

            xr = xT.rearrange("(kc p) (tc t) -> p kc tc t", p=128, t=512)
            bqk_r = bqk.rearrange("b (m p) -> b p m", p=128)

            # Two HW DGE queues issue in parallel (~650ns serialized per
            # dma_start per queue): x stream on sync, everything else on
            # the scalar queue in first-needed order.
            for tch in range(NT):
                nc.sync.dma_start(
                    out=xt_sb[:, :, tch * 512 : (tch + 1) * 512], in_=xr[:, :, tch]
                )
            nc.scalar.dma_start(out=bq_sb, in_=bqk_r[0])
            nc.scalar.dma_start(
                out=wq_sb, in_=wq.rearrange("(kc p) m -> p kc m", p=128)
            )
            nc.scalar.dma_start(
                out=wk_sb, in_=wk.rearrange("(kc p) m -> p kc m", p=128)
            )
            nc.scalar.dma_start(out=bk_sb, in_=bqk_r[1])
            nc.scalar.dma_start(
                out=wv_sb, in_=wv.rearrange("(kc p) m -> p kc m", p=128)
            )
            nc.scalar.dma_start(out=bv_sb, in_=bvr[:])
            nc.scalar.dma_start(out=id_sb, in_=ident[:])
            wexp_r = wexp.rearrange("pr h p w -> pr p h w")
            for p in range(NPAIR):
                nc.scalar.dma_start(out=ea_sb[p], in_=wexp_r[p])
            nc.scalar.dma_start(
                out=wp_sb, in_=wp.rearrange("(kc p) m -> p kc m", p=128)
            )

            # ---- persistent activations ----
            qT_sb = bigpool.tile([128, MC, T], FP16)  # c' = m*128 + p
            kT_sb = bigpool.tile([128, MC, T], FP16)
            v_sb = bigpool.tile([128, NK, HL * 65], FP16)  # slot l: [v(64), ones]
            y_sb = bigpool.tile([128, MC, T], FP16)  # y_cat_T, c_in = m*128 + p

            for l in range(HL):
                nc.vector.memset(v_sb[:, :, l * 65 + 64 : l * 65 + 65], 1.0)

            # ---- qkv / proj closures (PE fillers during attention) ----
            def qk_closure(tch, m, w_sb, b_sb, out_sb):
                def emit():
                    tsl = slice(tch * 512, (tch + 1) * 512)
                    msl = slice(m * 128, (m + 1) * 128)
                    pq = miscp.tile([128, 512], FP32, tag="misc")
                    for kc in range(KC):
                        nc.tensor.matmul(
                            pq[:],
                            w_sb[:, kc, msl],
                            xt_sb[:, kc, tsl],
                            start=(kc == 0),
                            stop=(kc == KC - 1),
                        )
                    nc.vector.tensor_scalar_add(
                        out=out_sb[:, m, tsl], in0=pq[:], scalar1=b_sb[:, m : m + 1]
                    )

                return emit

            def v_closure(tch, ts):
                def emit():
                    t16 = tch * 4 + ts
                    pv = miscp.tile([128, 512], FP32, tag="misc")
                    for kc in range(KC):
                        nc.tensor.matmul(
                            pv[:],
                            xt_sb[:, kc, t16 * 128 : (t16 + 1) * 128],
                            wv_sb[:, kc, :],
                            start=(kc == 0),
                            stop=(kc == KC - 1),
                        )
                    # scatter into 65-wide slots (even/odd strided copies) + bias
                    for par in range(2):
                        src = bass.AP(
                            tensor=pv.tensor,
                            offset=pv.offset + par * 64,
                            ap=[pv.ap[0], [128, 4], [1, 64]],
                        )
                        srcb = bass.AP(
                            tensor=bv_sb.tensor,
                            offset=bv_sb.offset + par * 64,
                            ap=[bv_sb.ap[0], [128, 4], [1, 64]],
                        )
                        base = v_sb[:, t16]
                        dst = bass.AP(
                            tensor=base.tensor,
                            offset=base.offset + par * 65,
                            ap=[base.ap[0], [130, 4], [1, 64]],
                        )
                        nc.vector.tensor_add(out=dst, in0=src, in1=srcb)

                return emit

            def proj_closure(tch, mo):
                def emit():
                    tsl = slice(tch * 512, (tch + 1) * 512)
                    osl = slice(mo * 128, (mo + 1) * 128)
                    pp = miscp.tile([128, 512], FP32, tag="misc")
                    for kcm in range(MC):
                        nc.tensor.matmul(
                            pp[:],
                            wp_sb[:, kcm, osl],
                            y_sb[:, kcm, tsl],
                            start=(kcm == 0),
                            stop=(kcm == MC - 1),
                        )
                    yo_sb = yopool.tile([128, 512], FP16, tag="yo")
                    nc.vector.tensor_copy(yo_sb[:], pp[:])
                    nc.sync.dma_start(out=yp[osl, tsl], in_=yo_sb[:])

                return emit

            def qkv_closures(tch):
                # q first (attention chunk tch needs qT before any kT tile)
                cl = [qk_closure(tch, m, wq_sb, bq_sb, qT_sb) for m in range(MC)]
                cl += [qk_closure(tch, m, wk_sb, bk_sb, kT_sb) for m in range(MC)]
                cl += [v_closure(tch, ts) for ts in range(4)]
                return cl

            # ---- attention emission, chunk-pipelined ----
            for cl in qkv_closures(0):
                cl()

            # Filler schedule: tail chunks are ACT-heavy, so push proj work
            # late (attn_3 gets proj_1+proj_2) and qkv_3 into attn_2.
            filler_lists = {
                0: qkv_closures(1),
                1: qkv_closures(2) + [proj_closure(0, mo) for mo in range(C // 128)],
                2: qkv_closures(3),
                3: [proj_closure(1, mo) for mo in range(C // 128)]
                + [proj_closure(2, mo) for mo in range(C // 128)],
            }

            for c in range(NT):
                fillers = deque(filler_lists[c])
                nj = 4 * c + 4
                total_iters = NPAIR * nj
                nfill = len(fillers)
                it = 0
                popped = 0

                for pair in range(NPAIR):
                    pavA = pavp.tile([128, 512], FP32, tag="pav")
                    pavB = pavp.tile([128, 512], FP32, tag="pav")
                    for j in range(nj):
                        off = max(0, 128 * j - 512 * c)  # multiple of 128
                        far = j <= 4 * c - 2
                        s = 512 * c - 128 * j + 384

                        pS = ps.tile([128, 1024], FP32, tag="pS")
                        nc.tensor.matmul(
                            pS[:, off:512],
                            kT_sb[0:64, pair, j * 128 : (j + 1) * 128],
                            qT_sb[0:64, pair, c * 512 + off : (c + 1) * 512],
                            start=True,
                            stop=True,
                        )
                        nc.tensor.matmul(
                            pS[:, 512 + off : 1024],
                            kT_sb[64:128, pair, j * 128 : (j + 1) * 128],
                            qT_sb[64:128, pair, c * 512 + off : (c + 1) * 512],
                            start=True,
                            stop=True,
                        )

                        # PE fillers go between S and AV so the tensor engine
                        # streams while ACT computes the exp
                        it += 1
                        due = nfill * it // total_iters
                        while popped < due:
                            fillers.popleft()()
                            popped += 1

                        p2 = p2pool.tile([128, 1024], FP16, tag="p2")
                        nc.scalar.activation(
                            out=p2[:, off:1024],
                            in_=pS[:, off:1024],
                            func=mybir.ActivationFunctionType.Exp,
                            scale=1.0 / math.sqrt(D),
                        )
                        if far:
                            pmt = p2
                        else:
                            pmt = pmpool.tile([128, 1024], FP16, tag="pm")
                            ea_pair = bass.AP(
                                tensor=ea_sb[pair].tensor,
                                offset=ea_sb[pair].offset + s + off,
                                ap=[ea_sb[pair].ap[0], [EA_W, 2], [1, 512 - off]],
                            )
                            dst = bass.AP(
                                tensor=pmt.tensor,
                                offset=pmt.offset + off,
                                ap=[pmt.ap[0], [512, 2], [1, 512 - off]],
                            )
                            srcp = bass.AP(
                                tensor=p2.tensor,
                                offset=p2.offset + off,
                                ap=[p2.ap[0], [512, 2], [1, 512 - off]],
                            )
                            nc.vector.tensor_mul(out=dst, in0=srcp, in1=ea_pair)

                        if DEBUG and pair == 0 and j == 4 * c + 1:
                            dps = smpool.tile([128, 1024], FP32, tag="dps")
                            nc.vector.tensor_copy(dps[:], pS[:])
                            nc.sync.dma_start(out=d_ps[c], in_=dps[:])
                            nc.sync.dma_start(out=d_pm[c], in_=pmt[:])

                        for k in range(off // 128, 4):
                            # start=True clears the has_written map of the
                            # WHOLE PSUM bank, and start=False overwrites
                            # where bits are clear — so only the bank's
                            # first matmul of the pair-chunk may set start.
                            # stop closes each subtile's group at its true
                            # last contribution (j == 4c+k; sim-only).
                            nc.tensor.matmul(
                                pavA[:, k * 65 : k * 65 + 65],
                                pmt[:, k * 128 : (k + 1) * 128],
                                v_sb[:, j, 2 * pair * 65 : 2 * pair * 65 + 65],
                                start=(j == 0 and k == 0),
                                stop=(j == 4 * c + k),
                                skip_group_check=True,
                            )
                            nc.tensor.matmul(
                                pavB[:, k * 65 : k * 65 + 65],
                                pmt[:, 512 + k * 128 : 512 + (k + 1) * 128],
                                v_sb[:, j, (2 * pair + 1) * 65 : (2 * pair + 1) * 65 + 65],
                                start=(j == 0 and k == 0),
                                stop=(j == 4 * c + k),
                                skip_group_check=True,
                            )

                    # ---- tail: normalize + transpose + evacuate ----
                    if DEBUG and pair == 0:
                        for hh, pv in ((0, pavA), (1, pavB)):
                            dpa = smpool.tile([128, 512], FP32, tag="dpa")
                            nc.vector.tensor_copy(dpa[:], pv[:])
                            nc.sync.dma_start(out=d_pav[c, hh], in_=dpa[:])
                    rec = smpool.tile([128, 8], FP32, tag="rec")
                    recA = bass.AP(
                        tensor=pavA.tensor,
                        offset=pavA.offset + 64,
                        ap=[pavA.ap[0], [65, 4]],
                    )
                    recB = bass.AP(
                        tensor=pavB.tensor,
                        offset=pavB.offset + 64,
                        ap=[pavB.ap[0], [65, 4]],
                    )
                    nc.vector.reciprocal(out=rec[:, 0:4], in_=recA)
                    nc.vector.reciprocal(out=rec[:, 4:8], in_=recB)
                    yn = smpool.tile([128, 4, 128], FP16, tag="yn")
                    for k in range(4):
                        nc.vector.tensor_scalar_mul(
                            out=yn[:, k, 0:64],
                            in0=pavA[:, k * 65 : k * 65 + 64],
                            scalar1=rec[:, k : k + 1],
                        )
                        nc.vector.tensor_scalar_mul(
                            out=yn[:, k, 64:128],
                            in0=pavB[:, k * 65 : k * 65 + 64],
                            scalar1=rec[:, 4 + k : 5 + k],
                        )
                    pT = miscp.tile([128, 512], FP16, tag="misc")
                    for k in range(4):
                        nc.tensor.transpose(
                            out=pT[:, k * 128 : (k + 1) * 128],
                            in_=yn[:, k],
                            identity=id_sb[:],
                        )
                    nc.vector.tensor_copy(
                        y_sb[:, pair, c * 512 : (c + 1) * 512], pT[:]
                    )

                while fillers:
                    fillers.popleft()()

            for mo in range(C // 128):
                proj_closure(NT - 1, mo)()

            if DEBUG:
                for m in range(MC):
                    nc.sync.dma_start(out=d_q[m], in_=qT_sb[:, m, :])
                    nc.sync.dma_start(out=d_k[m], in_=kT_sb[:, m, :])
                    nc.sync.dma_start(out=d_y[m], in_=y_sb[:, m, :])
                for t16 in range(NK):
                    nc.sync.dma_start(out=d_v[t16], in_=v_sb[:, t16, :])

    nc.compile()
    return nc


_NC = None
LAST_RESULTS = None


def _get_program():
    global _NC
    if _NC is None:
        _NC = _build_program()
    return _NC


# Bucket b covers distances d in [starts[b], starts[b+1]); verified bit-exact
# against the jax reference's _relative_position_bucket for T=2048.
_BUCKET_STARTS = np.array(
    [0, 1, 2, 3, 4, 5, 6, 7, 8, 9, 10, 11, 12, 13, 14, 15,
     16, 18, 20, 23, 26, 29, 33, 38, 43, 49, 55, 63, 72, 82, 93, 106]
)


def _rel_bias_buckets():
    """bucket(d) for d = q - k in [0, T)."""
    d = np.arange(T)
    return np.searchsorted(_BUCKET_STARTS, d, side="right") - 1


def _make_in_maps(x, W_attn, b_attn, W_proj, rel_emb):
    buckets = _rel_bias_buckets()  # [T]
    bias_by_dist = rel_emb[buckets, :]  # [T, H] fp32
    # Divide by exp(b31) per head: far tiles (all d >= 106, bucket 31) then
    # multiply by exactly 1.0 and can skip the mask-multiply; the factor
    # cancels in the softmax ratio.
    b31 = rel_emb[NUM_BUCKETS - 1, :]  # [H]
    # vec[h, j] = exp(bias[j - 511] - b31[h]) for j >= 511 else 0
    vec = np.zeros((H, EA_W + 127), dtype=np.float32)
    vec[:, 511 : 511 + T] = np.exp(bias_by_dist.T - b31[:, None])
    vec = vec.astype(np.float16)
    # expand to the per-head Toeplitz table A[h, p, x] = vec[h, x - p + 127]
    sw = np.lib.stride_tricks.sliding_window_view(vec, EA_W, axis=1)  # [H,128,EA_W]
    wexp_all = np.ascontiguousarray(sw[:, ::-1, :])  # [H, 128, EA_W]

    ident = np.eye(128, dtype=np.float16)

    in_maps = []
    for core in range(NCORES):
        b, hg = core // 2, core % 2
        csl = slice(hg * CL, (hg + 1) * CL)
        in_maps.append(
            {
                "xT": np.ascontiguousarray(x[b].T).astype(np.float16),
                "wq": np.ascontiguousarray(W_attn[csl, :].T).astype(np.float16),
                "wk": np.ascontiguousarray(
                    W_attn[C + hg * CL : C + (hg + 1) * CL, :].T
                ).astype(np.float16),
                "wv": np.ascontiguousarray(
                    W_attn[2 * C + hg * CL : 2 * C + (hg + 1) * CL, :].T
                ).astype(np.float16),
                "wp": np.ascontiguousarray(W_proj[:, csl].T).astype(np.float16),
                "bqk": np.stack(
                    [b_attn[csl], b_attn[C + hg * CL : C + (hg + 1) * CL]]
                ).astype(np.float32),
                "bvr": np.ascontiguousarray(
                    np.broadcast_to(
                        b_attn[2 * C + hg * CL : 2 * C + (hg + 1) * CL].astype(
                            np.float32
                        ),
                        (128, CL),
                    )
                ),
                "wexp": np.ascontiguousarray(
                    wexp_all[hg * HL : (hg + 1) * HL].reshape(NPAIR, 2, 128, EA_W)
                ),
                "ident": ident,
            }
        )
    return in_maps


def kernel(x, W_attn, b_attn, W_proj, b_proj, rel_emb):
    x = np.asarray(x)
    W_attn = np.asarray(W_attn)
    b_attn = np.asarray(b_attn)
    W_proj = np.asarray(W_proj)
    b_proj = np.asarray(b_proj)
    rel_emb = np.asarray(rel_emb)

    in_maps = _make_in_maps(x, W_attn, b_attn, W_proj, rel_emb)
    nc = _get_program()
    res = bass_utils.run_bass_kernel_spmd(nc, in_maps, core_ids=list(range(NCORES)))
    global LAST_RESULTS
    LAST_RESULTS = res

    y = np.empty((B, T, C), dtype=np.float32)
    for b in range(B):
        ypT = res.results[2 * b]["yp"].astype(np.float32) + res.results[2 * b + 1][
            "yp"
        ].astype(np.float32)
        y[b] = ypT.T + b_proj[None, :].astype(np.float32)
    return y
